# revision 1
# baseline (speedup 1.0000x reference)
"""Trainium2 Bass kernel for nn_AffineExpert (diag + rank-R linear recurrence).

Math: s_{t+1} = a_t*s_t + u_t + U (g_t * (V^T s_t)),  s_0 = 0, output s_S.
  a = sigmoid(x@Wa^T + ba), g = x@Wg^T + bg, u = x@Wu^T + bu.

Strategy per core (data-parallel over batch, 2 rows/core on 8 cores):
  * big projections as fp16 PE matmuls (fp32 PSUM accum), N=512 tiles,
    output layout [h-partition, t-free],
  * the recurrence is linear in the rank-R channel q_t = g_t*(V^T s_t);
    with q known, s is a pure diagonal-decay scan, done by the DVE
    tensor_tensor_scan instruction along t,
  * q itself is found by a fixed-point iteration (loop gain ~3%): per
    512-step chunk, scan -> project (V^T) -> q-update, 2 correction
    iterations + final scan give ~3e-4 relative error overall,
  * chunks chain exactly through the carried state column; chunk c+1
    projections (PE) overlap chunk c iterations (DVE) via Tile scheduling.
"""
import numpy as np

import concourse.bass as bass
import concourse.mybir as mybir
import concourse.tile as tile
from concourse import bacc
from concourse.bass_utils import run_bass_kernel_spmd

f32 = mybir.dt.float32
f16 = mybir.dt.float16
AF = mybir.ActivationFunctionType
OP = mybir.AluOpType

B, S, D, H, R = 16, 2048, 1024, 1024, 16
N_CORES = 8
B_CORE = B // N_CORES
CHUNK = 512
# scans per chunk = K_INNER + 1 (z0 + (K_INNER-1) corrections + final).
# K_INNER=2 verified in numpy: iteration error ~5e-5, below the fp16
# matmul noise floor (~3e-4); K_INNER=3 measured identical accuracy.
K_INNER = 2


def build_kernel(B_core=B_CORE, S_=S, D_=D, H_=H, R_=R, C=CHUNK, k_inner=K_INNER):
    KC, HC, NCH = D_ // 128, H_ // 128, S_ // C
    # Bacc (not raw Bass): its compile() pass legalizes semaphore waits
    # (1-wait-per-instruction hardware limit) via event semaphores.
    nc = bacc.Bacc("TRN2")

    xT = nc.dram_tensor("xT", [B_core, KC, 128, S_], f32, kind="ExternalInput")
    waT = nc.dram_tensor("waT", [KC, 128, H_], f32, kind="ExternalInput")
    wuT = nc.dram_tensor("wuT", [KC, 128, H_], f32, kind="ExternalInput")
    wgT = nc.dram_tensor("wgT", [KC, 128, R_], f32, kind="ExternalInput")
    uT_d = nc.dram_tensor("uT", [R_, H_], f32, kind="ExternalInput")
    v_d = nc.dram_tensor("v", [H_, R_], f32, kind="ExternalInput")
    ba_d = nc.dram_tensor("ba", [128, HC], f32, kind="ExternalInput")
    bu_d = nc.dram_tensor("bu", [128, HC], f32, kind="ExternalInput")
    bg_d = nc.dram_tensor("bg", [R_, 1], f32, kind="ExternalInput")
    out_d = nc.dram_tensor("out", [B_core, H_], f32, kind="ExternalOutput")

    with tile.TileContext(nc) as tc:
        with tc.tile_pool(name="persist", bufs=1) as persist, \
             tc.tile_pool(name="work", bufs=2) as work, \
             tc.tile_pool(name="trans", bufs=3) as trans, \
             tc.tile_pool(name="upool", bufs=8) as upool, \
             tc.tile_pool(name="ps_big", bufs=4, space="PSUM") as ps_big, \
             tc.tile_pool(name="ps_p", bufs=2, space="PSUM") as ps_p, \
             tc.tile_pool(name="ps_small", bufs=2, space="PSUM") as ps_small:

            # ---------- startup: stage weights to fp16, init state ----------
            w16a = persist.tile([128, KC, H_], f16)
            w16u = persist.tile([128, KC, H_], f16)
            wg16 = persist.tile([128, KC, R_], f16)
            v16 = persist.tile([128, HC, R_], f16)
            u16T = persist.tile([R_, H_], f16)
            ba_t = persist.tile([128, HC], f32)
            bu_t = persist.tile([128, HC], f32)
            bg_t = persist.tile([R_, 1], f32)
            state_cols = persist.tile([128, B_core * HC], f32)
            p_first = persist.tile([R_, B_core], f32)

            # gpsimd SWDGE casts fp32->fp16 in transit; no landing tiles, so
            # every staging DMA carries at most one wait.
            for kc in range(KC):
                nc.gpsimd.dma_start(wg16[:, kc, :], wgT[kc])
            for kc in range(KC):
                nc.gpsimd.dma_start(w16a[:, kc, :], waT[kc])
            for kc in range(KC):
                nc.gpsimd.dma_start(w16u[:, kc, :], wuT[kc])
            nc.gpsimd.dma_start(
                v16[:], v_d.rearrange("(hc p) r -> p hc r", p=128))
            nc.gpsimd.dma_start(u16T[:], uT_d[:, :])
            nc.sync.dma_start(ba_t[:], ba_d[:, :])
            nc.sync.dma_start(bu_t[:], bu_d[:, :])
            nc.sync.dma_start(bg_t[:], bg_d[:, :])

            nc.vector.memset(state_cols[:], 0.0)
            nc.vector.memset(p_first[:], 0.0)

            # ---------- chunk loop ----------
            for c in range(NCH):
                t0 = c * C
                x16 = {}
                for row in range(B_core):
                    for kc in range(KC):
                        xt = work.tile([128, C], f16, tag=f"x_{row}_{kc}")
                        nc.gpsimd.dma_start(
                            xt[:], xT[row, kc, :, t0:t0 + C])
                        x16[row, kc] = xt

                a_tiles = {}
                g_tiles = {}
                zl_tiles = {}
                p0s_tiles = {}
                for row in range(B_core):
                    # g projection [R, C]
                    gp = ps_small.tile([R_, C], f32, tag="small")
                    for kc in range(KC):
                        nc.tensor.matmul(
                            gp[:], wg16[:, kc, :], x16[row, kc][:],
                            start=(kc == 0), stop=(kc == KC - 1))
                    gt = work.tile([R_, C], f32, tag=f"g_{row}")
                    nc.scalar.activation(gt[:], gp[:], AF.Identity, bias=bg_t[:])
                    g_tiles[row] = gt

                    zlast = work.tile([128, HC], f32, tag=f"zl_{row}")
                    zl_tiles[row] = zlast
                    p0p = ps_p.tile([R_, C], f32, tag="pp")
                    for hc in range(HC):
                        hs = slice(hc * 128, (hc + 1) * 128)
                        ap = ps_big.tile([128, C], f32, tag="big")
                        for kc in range(KC):
                            nc.tensor.matmul(
                                ap[:], w16a[:, kc, hs], x16[row, kc][:],
                                start=(kc == 0), stop=(kc == KC - 1))
                        at = work.tile([128, C], f32, tag=f"a_{row}_{hc}")
                        nc.scalar.activation(
                            at[:], ap[:], AF.Sigmoid, bias=ba_t[:, hc:hc + 1])
                        a_tiles[row, hc] = at

                        up = ps_big.tile([128, C], f32, tag="big")
                        for kc in range(KC):
                            nc.tensor.matmul(
                                up[:], w16u[:, kc, hs], x16[row, kc][:],
                                start=(kc == 0), stop=(kc == KC - 1))
                        ut = upool.tile([128, C], f32, tag="ut")
                        nc.scalar.activation(
                            ut[:], up[:], AF.Identity, bias=bu_t[:, hc:hc + 1])

                        # z0 scan: state column as initial
                        col = row * HC + hc
                        z0 = trans.tile([128, C], f32, tag="z0")
                        nc.vector.tensor_tensor_scan(
                            z0[:], at[:], ut[:],
                            state_cols[:, col:col + 1], OP.mult, OP.add)
                        nc.vector.tensor_copy(
                            zlast[:, col - row * HC:col - row * HC + 1],
                            z0[:, C - 1:C])
                        z16 = trans.tile([128, C], f16, tag="z16")
                        nc.vector.tensor_copy(z16[:], z0[:])
                        nc.tensor.matmul(
                            p0p[:], v16[:, hc, :], z16[:],
                            start=(hc == 0), stop=(hc == HC - 1))

                    p0s = work.tile([R_, C], f32, tag=f"p0s_{row}")
                    nc.vector.tensor_copy(p0s[:], p0p[:])
                    p0s_tiles[row] = p0s

                # ---------- fixed-point iterations ----------
                q16 = {}
                for row in range(B_core):
                    pf = work.tile([R_, C], f32, tag=f"pf_{row}")
                    nc.vector.tensor_copy(
                        pf[:, 0:1], p_first[:, row:row + 1])
                    nc.vector.tensor_copy(
                        pf[:, 1:C], p0s_tiles[row][:, 0:C - 1])
                    qt = work.tile([R_, C], f16, tag=f"q_{row}")
                    nc.vector.tensor_tensor(
                        qt[:], g_tiles[row][:], pf[:], OP.mult)
                    q16[row] = qt

                for j in range(1, k_inner):
                    for row in range(B_core):
                        ppj = ps_p.tile([R_, C], f32, tag="pp")
                        for hc in range(HC):
                            uqp = ps_big.tile([128, C], f32, tag="big")
                            nc.tensor.matmul(
                                uqp[:],
                                u16T[:, hc * 128:(hc + 1) * 128],
                                q16[row][:], start=True, stop=True)
                            c16 = trans.tile([128, C], f16, tag="c16")
                            nc.vector.tensor_tensor_scan(
                                c16[:], a_tiles[row, hc][:], uqp[:],
                                0.0, OP.mult, OP.add)
                            nc.tensor.matmul(
                                ppj[:], v16[:, hc, :], c16[:],
                                start=(hc == 0), stop=(hc == HC - 1))
                        pf = work.tile([R_, C], f32, tag=f"pf_{row}")
                        nc.vector.tensor_copy(
                            pf[:, 0:1], p_first[:, row:row + 1])
                        nc.vector.tensor_tensor(
                            pf[:, 1:C], p0s_tiles[row][:, 0:C - 1],
                            ppj[:, 0:C - 1], OP.add)
                        qt = work.tile([R_, C], f16, tag=f"q_{row}")
                        nc.vector.tensor_tensor(
                            qt[:], g_tiles[row][:], pf[:], OP.mult)
                        q16[row] = qt

                # ---------- final scan: state + next p_first ----------
                for row in range(B_core):
                    pfp = ps_small.tile([R_, C], f32, tag="small")
                    for hc in range(HC):
                        uqp = ps_big.tile([128, C], f32, tag="big")
                        nc.tensor.matmul(
                            uqp[:], u16T[:, hc * 128:(hc + 1) * 128],
                            q16[row][:], start=True, stop=True)
                        c16 = trans.tile([128, C], f16, tag="c16")
                        nc.vector.tensor_tensor_scan(
                            c16[:], a_tiles[row, hc][:], uqp[:],
                            0.0, OP.mult, OP.add)
                        col = row * HC + hc
                        nc.vector.tensor_tensor(
                            state_cols[:, col:col + 1],
                            zl_tiles[row][:, hc:hc + 1],
                            c16[:, C - 1:C], OP.add)
                        nc.tensor.matmul(
                            pfp[:, 0:1], v16[:, hc, :], c16[:, C - 1:C],
                            start=(hc == 0), stop=(hc == HC - 1))
                    nc.vector.tensor_tensor(
                        p_first[:, row:row + 1],
                        p0s_tiles[row][:, C - 1:C],
                        pfp[:, 0:1], OP.add)

            # ---------- output ----------
            for row in range(B_core):
                for hc in range(HC):
                    col = row * HC + hc
                    nc.sync.dma_start(
                        out_d[row, hc * 128:(hc + 1) * 128],
                        state_cols[:, col:col + 1])
    nc.finalize()
    return nc


def make_in_maps(x, Wa, ba, Wg, bg, Wu, bu, u, v, n_cores=N_CORES):
    """Shard + lay out host-side (pure layout transforms, fp32 kept)."""
    B_, S_, D_ = x.shape
    H_, R_ = u.shape
    KC, HC = D_ // 128, H_ // 128
    b_core = B_ // n_cores
    waT = np.ascontiguousarray(Wa.T).reshape(KC, 128, H_)
    wuT = np.ascontiguousarray(Wu.T).reshape(KC, 128, H_)
    wgT = np.ascontiguousarray(Wg.T).reshape(KC, 128, R_)
    uT = np.ascontiguousarray(u.T)
    ba_h = np.ascontiguousarray(ba.reshape(HC, 128).T)
    bu_h = np.ascontiguousarray(bu.reshape(HC, 128).T)
    bg_h = np.ascontiguousarray(bg.reshape(R_, 1))
    in_maps = []
    for core in range(n_cores):
        rows = slice(core * b_core, (core + 1) * b_core)
        xT = np.ascontiguousarray(
            x[rows].transpose(0, 2, 1)).reshape(b_core, KC, 128, S_)
        in_maps.append({
            "xT": xT, "waT": waT, "wuT": wuT, "wgT": wgT, "uT": uT,
            "v": np.ascontiguousarray(v), "ba": ba_h, "bu": bu_h, "bg": bg_h,
        })
    return in_maps


def kernel(x, Wa, ba, Wg, bg, Wu, bu, u, v):
    x = np.asarray(x, dtype=np.float32)
    in_maps = make_in_maps(
        x, np.asarray(Wa), np.asarray(ba), np.asarray(Wg), np.asarray(bg),
        np.asarray(Wu), np.asarray(bu), np.asarray(u), np.asarray(v))
    nc = build_kernel()
    res = run_bass_kernel_spmd(nc, in_maps, core_ids=list(range(N_CORES)))
    return np.concatenate(
        [res.results[i]["out"] for i in range(N_CORES)], axis=0)


if __name__ == "__main__":
    import reference  # only when run manually next to reference.py

    inputs = {k: np.asarray(v) for k, v in reference.setup_inputs().items()}
    got = kernel(**inputs)
    exp = np.asarray(reference.reference(**inputs))
    print("relmax:", np.abs(got - exp).max() / np.abs(exp).max())



# revision 2
# speedup vs baseline: 1.3526x; 1.3526x over previous
"""Trainium2 Bass kernel for nn_AffineExpert (diag + rank-R linear recurrence).

Math: s_{t+1} = a_t*s_t + u_t + U (g_t * (V^T s_t)),  s_0 = 0, output s_S.
  a = sigmoid(x@Wa^T + ba), g = x@Wg^T + bg, u = x@Wu^T + bu.

Strategy per core (data-parallel over batch, 2 rows/core on 8 cores):

  * All heavy projections (a, u, g) are fp16 PE matmuls with fp32 PSUM
    accumulation, N=512 time-tiles; inputs staged to fp16 on the host so
    DMA moves half the bytes and no on-device cast is needed.
  * The recurrence is linear in the rank-R channel q_t = g_t*(V^T s_t).
    Per 512-step chunk: z0 = diag-decay scan of (a, u) from the carried
    state (DVE tensor_tensor_scan), p0 = V^T z0, q = g * shift(p0), then
    one more scan of (a, U q) gives the low-rank correction.  One
    fixed-point pass (no inner correction iteration) keeps the final
    error ~1.5e-3, well under the 2e-2 gate.
  * Chunks chain through the carried fp16 state columns; q's first step
    uses p_first = V^T s_final of the previous chunk (computed by tiny
    N=1 matmuls on the updated state).
  * Software pipelining: projection matmuls of chunk c+1 are emitted
    interleaved with the recurrence phase of chunk c, so the PE always
    has dense independent work while the DVE runs the serial scan chain
    (keeps the PE HAM-warm at 2.4 GHz; the old version oscillated cold).
"""
import numpy as np

import concourse.bass as bass
import concourse.mybir as mybir
import concourse.tile as tile
from concourse import bacc
from concourse.bass_utils import run_bass_kernel_spmd

f32 = mybir.dt.float32
f16 = mybir.dt.float16
AF = mybir.ActivationFunctionType
OP = mybir.AluOpType

B, S, D, H, R = 16, 2048, 1024, 1024, 16
N_CORES = 8
B_CORE = B // N_CORES
CHUNK = 512


def build_kernel(B_core=B_CORE, S_=S, D_=D, H_=H, R_=R, C=CHUNK):
    KC, HC, NCH = D_ // 128, H_ // 128, S_ // C
    nc = bacc.Bacc("TRN2")

    # host-prepped, chunk-contiguous fp16 x: [NCH, B_core, KC, 128, C]
    xc = nc.dram_tensor("xc", [NCH, B_core, KC, 128, C], f16, kind="ExternalInput")
    waT = nc.dram_tensor("waT", [KC, 128, H_], f16, kind="ExternalInput")
    wuT = nc.dram_tensor("wuT", [KC, 128, H_], f16, kind="ExternalInput")
    wgT = nc.dram_tensor("wgT", [KC, 128, R_], f16, kind="ExternalInput")
    uT_d = nc.dram_tensor("uT", [R_, H_], f16, kind="ExternalInput")
    v_d = nc.dram_tensor("v", [128, HC, R_], f16, kind="ExternalInput")
    ba_d = nc.dram_tensor("ba", [128, HC], f32, kind="ExternalInput")
    bu_d = nc.dram_tensor("bu", [128, HC], f32, kind="ExternalInput")
    bg_d = nc.dram_tensor("bg", [R_, 1], f32, kind="ExternalInput")
    out_d = nc.dram_tensor("out", [B_core, H_], f16, kind="ExternalOutput")

    with tile.TileContext(nc) as tc:
        with tc.tile_pool(name="persist", bufs=1) as persist, \
             tc.tile_pool(name="xpool", bufs=2) as xpool, \
             tc.tile_pool(name="apool", bufs=2) as apool, \
             tc.tile_pool(name="upool", bufs=2) as upool, \
             tc.tile_pool(name="zpool", bufs=2) as zpool, \
             tc.tile_pool(name="spool", bufs=2) as spool, \
             tc.tile_pool(name="clpool", bufs=4) as clpool, \
             tc.tile_pool(name="ps_proj", bufs=3, space="PSUM") as ps_proj, \
             tc.tile_pool(name="ps_uq", bufs=2, space="PSUM") as ps_uq, \
             tc.tile_pool(name="ps_p", bufs=2, space="PSUM") as ps_p, \
             tc.tile_pool(name="ps_tiny", bufs=1, space="PSUM") as ps_tiny:

            # ---------- persistent staging ----------
            wa16 = persist.tile([128, KC, H_], f16)
            wu16 = persist.tile([128, KC, H_], f16)
            wg16 = persist.tile([128, KC, R_], f16)
            v16 = persist.tile([128, HC, R_], f16)
            u16T = persist.tile([R_, H_], f16)
            ba_t = persist.tile([128, HC], f32)
            bu_t = persist.tile([128, HC], f32)
            bg_t = persist.tile([R_, 1], f32)
            state16 = persist.tile([128, B_core * HC], f16)
            p_first = persist.tile([R_, B_core], f32)

            for kc in range(KC):
                nc.sync.dma_start(wg16[:, kc, :], wgT[kc])
            for kc in range(KC):
                nc.sync.dma_start(wa16[:, kc, :], waT[kc])
            for kc in range(KC):
                nc.sync.dma_start(wu16[:, kc, :], wuT[kc])
            nc.sync.dma_start(v16[:], v_d[:, :, :])
            nc.sync.dma_start(u16T[:], uT_d[:, :])
            nc.sync.dma_start(ba_t[:], ba_d[:, :])
            nc.sync.dma_start(bu_t[:], bu_d[:, :])
            nc.sync.dma_start(bg_t[:], bg_d[:, :])

            nc.vector.memset(state16[:], 0.0)
            nc.vector.memset(p_first[:], 0.0)

            # ---------- emission helpers ----------
            x16 = {}     # (row, kc) -> live x tile of the chunk being projected
            a16 = {}     # (row, hc) -> sigmoid activations
            u16 = {}     # (row, hc)
            g16 = {}     # row
            z0t = {}     # (row, hc) -> z0 scan output (kept for last column)

            def emit_x_dma(c):
                for row in range(B_core):
                    for kc in range(KC):
                        xt = xpool.tile([128, C], f16, tag=f"x_{row}_{kc}")
                        nc.sync.dma_start(xt[:], xc[c, row, kc])
                        x16[row, kc] = xt

            def emit_g(row):
                gp = ps_proj.tile([R_, C], f32, tag="proj")
                for kc in range(KC):
                    nc.tensor.matmul(
                        gp[:], wg16[:, kc, :], x16[row, kc][:],
                        start=(kc == 0), stop=(kc == KC - 1))
                gt = spool.tile([R_, C], f16, tag=f"g_{row}")
                nc.scalar.activation(gt[:], gp[:], AF.Identity, bias=bg_t[:])
                g16[row] = gt

            def emit_a(row, hc):
                hs = slice(hc * 128, (hc + 1) * 128)
                ap = ps_proj.tile([128, C], f32, tag="proj")
                for kc in range(KC):
                    nc.tensor.matmul(
                        ap[:], wa16[:, kc, hs], x16[row, kc][:],
                        start=(kc == 0), stop=(kc == KC - 1))
                at = apool.tile([128, C], f16, tag=f"a_{row}_{hc}")
                nc.scalar.activation(
                    at[:], ap[:], AF.Sigmoid, bias=ba_t[:, hc:hc + 1])
                a16[row, hc] = at

            def emit_u(row, hc):
                hs = slice(hc * 128, (hc + 1) * 128)
                up = ps_proj.tile([128, C], f32, tag="proj")
                for kc in range(KC):
                    nc.tensor.matmul(
                        up[:], wu16[:, kc, hs], x16[row, kc][:],
                        start=(kc == 0), stop=(kc == KC - 1))
                ut = upool.tile([128, C], f16, tag=f"u_{row}_{hc}")
                nc.scalar.activation(
                    ut[:], up[:], AF.Identity, bias=bu_t[:, hc:hc + 1])
                u16[row, hc] = ut

            # S(c) phase A for one row: z0 scans + V^T z0 + q build.
            def emit_SA(row):
                p0p = ps_p.tile([R_, C], f32, tag="p0")
                for hc in range(HC):
                    col = row * HC + hc
                    z0 = zpool.tile([128, C], f16, tag=f"z_{row}_{hc}")
                    nc.vector.tensor_tensor_scan(
                        z0[:], a16[row, hc][:], u16[row, hc][:],
                        state16[:, col:col + 1], OP.mult, OP.add)
                    z0t[row, hc] = z0
                    nc.tensor.matmul(
                        p0p[:], v16[:, hc, :], z0[:],
                        start=(hc == 0), stop=(hc == HC - 1))
                qt = spool.tile([R_, C], f16, tag=f"q_{row}")
                nc.vector.tensor_tensor(
                    qt[:, 1:C], g16[row][:, 1:C], p0p[:, 0:C - 1], OP.mult)
                nc.vector.tensor_tensor(
                    qt[:, 0:1], g16[row][:, 0:1], p_first[:, row:row + 1],
                    OP.mult)
                return qt

            # S(c) phase B for one row: Uq + correction scans + state update
            # + p_first refresh.
            def emit_SB(row, qt):
                for hc in range(HC):
                    hs = slice(hc * 128, (hc + 1) * 128)
                    col = row * HC + hc
                    uqp = ps_uq.tile([128, C], f32, tag="uq")
                    nc.tensor.matmul(
                        uqp[:], u16T[:, hs], qt[:], start=True, stop=True)
                    cl = clpool.tile([128, C], f16, tag="cl")
                    nc.vector.tensor_tensor_scan(
                        cl[:], a16[row, hc][:], uqp[:], 0.0, OP.mult, OP.add)
                    nc.vector.tensor_tensor(
                        state16[:, col:col + 1], z0t[row, hc][:, C - 1:C],
                        cl[:, C - 1:C], OP.add)
                pfp = ps_tiny.tile([R_, 1], f32, tag="pf")
                for hc in range(HC):
                    col = row * HC + hc
                    nc.tensor.matmul(
                        pfp[:], v16[:, hc, :], state16[:, col:col + 1],
                        start=(hc == 0), stop=(hc == HC - 1))
                nc.vector.tensor_copy(p_first[:, row:row + 1], pfp[:])

            # ---------- pipelined chunk loop ----------
            # prologue: projections of chunk 0
            emit_x_dma(0)
            for row in range(B_core):
                emit_g(row)
            for hc in range(HC):
                for row in range(B_core):
                    emit_a(row, hc)
                for row in range(B_core):
                    emit_u(row, hc)

            for c in range(NCH):
                last = (c == NCH - 1)
                if not last:
                    emit_x_dma(c + 1)
                # Phase A of S(c), interleaved with a-projections of c+1.
                qts = {}
                for row in range(B_core):
                    qts[row] = emit_SA(row)
                    if not last:
                        emit_g(row)
                        for hc in range(HC):
                            emit_a(row, hc)
                # Phase B of S(c), interleaved with u-projections of c+1.
                for row in range(B_core):
                    emit_SB(row, qts[row])
                    if not last:
                        for hc in range(HC):
                            emit_u(row, hc)

            # ---------- output ----------
            for row in range(B_core):
                for hc in range(HC):
                    col = row * HC + hc
                    nc.sync.dma_start(
                        out_d[row, hc * 128:(hc + 1) * 128],
                        state16[:, col:col + 1])
    nc.finalize()
    return nc


def make_in_maps(x, Wa, ba, Wg, bg, Wu, bu, u, v, n_cores=N_CORES, C=CHUNK):
    """Shard + lay out host-side (layout transforms + fp16 casts)."""
    B_, S_, D_ = x.shape
    H_, R_ = u.shape
    KC, HC, NCH = D_ // 128, H_ // 128, S_ // C
    b_core = B_ // n_cores
    waT = np.ascontiguousarray(Wa.T).reshape(KC, 128, H_).astype(np.float16)
    wuT = np.ascontiguousarray(Wu.T).reshape(KC, 128, H_).astype(np.float16)
    wgT = np.ascontiguousarray(Wg.T).reshape(KC, 128, R_).astype(np.float16)
    uT = np.ascontiguousarray(u.T).astype(np.float16)
    vh = np.ascontiguousarray(
        v.reshape(HC, 128, R_).transpose(1, 0, 2)).astype(np.float16)
    ba_h = np.ascontiguousarray(ba.reshape(HC, 128).T).astype(np.float32)
    bu_h = np.ascontiguousarray(bu.reshape(HC, 128).T).astype(np.float32)
    bg_h = np.ascontiguousarray(bg.reshape(R_, 1)).astype(np.float32)
    in_maps = []
    for core in range(n_cores):
        rows = slice(core * b_core, (core + 1) * b_core)
        # [b, S, D] -> [NCH, b, KC, 128, C], fully contiguous per tile
        xcore = x[rows].astype(np.float16)
        xck = xcore.reshape(b_core, NCH, C, KC, 128)
        xc = np.ascontiguousarray(xck.transpose(1, 0, 3, 4, 2))
        in_maps.append({
            "xc": xc, "waT": waT, "wuT": wuT, "wgT": wgT, "uT": uT,
            "v": vh, "ba": ba_h, "bu": bu_h, "bg": bg_h,
        })
    return in_maps


def kernel(x, Wa, ba, Wg, bg, Wu, bu, u, v):
    x = np.asarray(x, dtype=np.float32)
    in_maps = make_in_maps(
        x, np.asarray(Wa), np.asarray(ba), np.asarray(Wg), np.asarray(bg),
        np.asarray(Wu), np.asarray(bu), np.asarray(u), np.asarray(v))
    nc = build_kernel()
    res = run_bass_kernel_spmd(nc, in_maps, core_ids=list(range(N_CORES)))
    return np.concatenate(
        [res.results[i]["out"].astype(np.float32) for i in range(N_CORES)],
        axis=0)


if __name__ == "__main__":
    import reference  # only when run manually next to reference.py

    inputs = {k: np.asarray(v) for k, v in reference.setup_inputs().items()}
    got = kernel(**inputs)
    exp = np.asarray(reference.reference(**inputs))
    print("relmax:", np.abs(got - exp).max() / np.abs(exp).max())
